# revision 12
# baseline (speedup 1.0000x reference)
"""Multi-head attention kernel for Trainium2 (Bass/Tile), 8-core data-parallel.

Problem: B=1024 batches of F=128 tokens, D=128 features, H=8 heads, dh=16.
  out = softmax(X Wq (X Wk)^T / sqrt(D)) (X Wv) + X Wr   (per head, concat)

Per-core structure (128 batches):
  - Host pre-transposes X to XT [D, B, F] bf16 (no on-device transpose,
    halved input DMA). All matmuls bf16 (fp32 matmul is 4x slower on PE).
  - Wq/Wk host-padded into A (heads 0-3) / B (heads 4-7) tiles with each
    head's 16 cols at a 32-aligned offset + 16 zero cols -> score matmuls
    are K=32 row-tiled matmuls at legal 32-aligned bases.
  - PSUM same-bank rule: matmuls with different row bases must not share a
    PSUM bank. Scores go to two 2-bank tiles X (bases 0/32 = head pairs
    {0,4},{1,5}) and Y (bases 64/96 = {2,6},{3,7}), one base per bank.
  - exp on ScalarE: two strided [128,512] instructions per batch (X then Y),
    scale=1/sqrt(D) fused, bf16 out feeds attn@V as stationary operand.
    Softmax max-subtraction skipped (|scores|/sqrt(D) < ~1).
  - attn@V: lhsT = expT_h [k,q], rhs = V'_h (16 value cols + 1 ones col) ->
    matmul emits unnormalized output AND the softmax denominator.
  - V'(+ones slot), R, and attn output share one PSUM bank per batch
    (all base-0 matmuls); pair-level tile for 2 batches (2 banks).
  - Tail on VectorE at pair granularity: recip(denoms), broadcast-expand
    (GpSimd), out = attn_unnorm * recip + R.
  - Output staged [F, B, E]; host transposes back.
"""

import numpy as np
import ml_dtypes

import concourse.bass as bass
import concourse.mybir as mybir
import concourse.tile as tile
from concourse import bacc
from concourse.bass_utils import run_bass_kernel_spmd

BF16 = ml_dtypes.bfloat16

N_CORES = 8
B, F, D = 1024, 128, 128
H, DH = 8, 16
BPC = B // N_CORES   # 128 batches per core
GIO = 8              # batches per IO wave (DMA granularity)
PAIR = 2             # batches per projection/tail pair
VCOLS = H * (DH + 1)  # 136
VRA = VCOLS + D + VCOLS  # 408: [V' 0:136 | R 136:264 | attn-out 264:400]
SCALE = 1.0 / float(D) ** 0.5
# et column-block order: X tile = heads 0,4,1,5; Y tile = heads 2,6,3,7
HORD = [0, 4, 1, 5, 2, 6, 3, 7]


def build_kernel(nc: bass.Bass):
    f32 = mybir.dt.float32
    bf16 = mybir.dt.bfloat16

    xt = nc.dram_tensor("xt", [D, BPC, F], bf16, kind="ExternalInput")
    # [WqA | WqB | WkA | WkB], each [D, 128], heads at 32-aligned cols
    wqk = nc.dram_tensor("wqk", [D, 4 * D], bf16, kind="ExternalInput")
    # [WvPad (136 cols, ones-slots zero) | Wr (128)]
    wvr = nc.dram_tensor("wvr", [D, VCOLS + D], bf16, kind="ExternalInput")
    out = nc.dram_tensor("out", [F, BPC, D], f32, kind="ExternalOutput")

    with tile.TileContext(nc) as tc:
        with (
            tc.tile_pool(name="singles", bufs=1) as singles,
            tc.tile_pool(name="xtp", bufs=2) as xtp,
            tc.tile_pool(name="qksb", bufs=2) as qksb,
            tc.tile_pool(name="etp", bufs=2) as etp,
            tc.tile_pool(name="vp", bufs=2) as vp,
            tc.tile_pool(name="smalls", bufs=2) as smalls,
            tc.tile_pool(name="outp", bufs=2) as outp,
            tc.tile_pool(name="qkps", bufs=2, space="PSUM") as qkps_pool,
            tc.tile_pool(name="scxp", bufs=1, space="PSUM") as scxp_pool,
            tc.tile_pool(name="scyp", bufs=1, space="PSUM") as scyp_pool,
            tc.tile_pool(name="vraps", bufs=1, space="PSUM") as vraps_pool,
        ):
            wqk_sb = singles.tile([D, 4 * D], bf16)
            wvr_sb = singles.tile([D, VCOLS + D], bf16)
            nc.sync.dma_start(out=wqk_sb, in_=wqk[:, :])
            nc.sync.dma_start(out=wvr_sb, in_=wvr[:, :])

            for w in range(BPC // GIO):  # 16 IO waves
                xtw = xtp.tile([D, GIO * F], bf16)
                nc.sync.dma_start(out=xtw, in_=xt[:, w * GIO:(w + 1) * GIO, :])
                ow = outp.tile([F, GIO * D], f32)

                for p in range(GIO // PAIR):  # 4 pairs per wave
                    # ---- QT/KT per batch: [QA|QB|KA|KB] x 128 cols each ----
                    qk_sbs = []
                    for b in range(PAIR):
                        gb = p * PAIR + b
                        xtb = xtw[:, gb * F:(gb + 1) * F]
                        qk_ps = qkps_pool.tile([D, 4 * F], f32)  # 1 bank
                        for i in range(4):
                            nc.tensor.matmul(
                                qk_ps[:, i * F:(i + 1) * F],
                                lhsT=wqk_sb[:, i * D:(i + 1) * D],
                                rhs=xtb,
                                start=True, stop=True,
                            )
                        qk_sb = qksb.tile([D, 4 * F], bf16)
                        nc.vector.tensor_copy(qk_sb, qk_ps)
                        qk_sbs.append(qk_sb)

                    # ---- V'/R projections for both batches of the pair ----
                    # [128, 1024] = 2 banks; batch b at cols 512*b + [0:400]
                    vra_ps = vraps_pool.tile([F, 2 * 512], f32)
                    for b in range(PAIR):
                        gb = p * PAIR + b
                        xtb = xtw[:, gb * F:(gb + 1) * F]
                        nc.tensor.matmul(
                            vra_ps[:, b * 512:b * 512 + VCOLS + D],
                            lhsT=xtb, rhs=wvr_sb, start=True, stop=True,
                        )
                    v_sb = vp.tile([F, PAIR * VCOLS], bf16)
                    v4 = v_sb.rearrange("p (b h c) -> p b h c", b=PAIR, c=DH + 1)
                    vrav = bass.AP(
                        tensor=vra_ps.tensor, offset=vra_ps.offset,
                        ap=[vra_ps.ap[0], [512, PAIR], [DH + 1, H], [1, DH]],
                    )
                    nc.gpsimd.memset(v4[:, :, :, DH:DH + 1], 1.0)
                    nc.vector.tensor_copy(v4[:, :, :, 0:DH], vrav)
                    rc_pair = smalls.tile([F, PAIR * H], f32, tag="rc")

                    for b in range(PAIR):
                        gb = p * PAIR + b
                        qk_sb = qk_sbs[b]
                        qtA = qk_sb[:, 0 * F:1 * F]
                        qtB = qk_sb[:, 1 * F:2 * F]
                        ktA = qk_sb[:, 2 * F:3 * F]
                        ktB = qk_sb[:, 3 * F:4 * F]

                        # ---- scores into X (bases 0/32), Y (bases 64/96);
                        # emit all X heads, then Y, so exp-X can start while
                        # PE fills Y ----
                        sc_x = scxp_pool.tile([F, 1024], f32)  # 2 banks
                        sc_y = scyp_pool.tile([F, 1024], f32)  # 2 banks
                        for h in HORD:
                            qt = qtA if h < 4 else qtB
                            kt = ktA if h < 4 else ktB
                            s = (h % 4) * 32
                            sc = sc_x if (h % 4) < 2 else sc_y
                            col = ((h % 4) % 2) * 512 + (h // 4) * F
                            nc.tensor.matmul(
                                sc[:, col:col + F],
                                lhsT=kt[s:s + 32, :],
                                rhs=qt[s:s + 32, :],
                                start=True, stop=True,
                                tile_position=(s, 0),
                            )

                        # ---- exp: one strided instr per sc tile ----
                        et_sb = etp.tile([F, H * F], bf16)
                        for t_i, sc in enumerate((sc_x, sc_y)):
                            sc3 = sc.rearrange("p (bk c) -> p bk c", bk=2)
                            nc.scalar.activation(
                                et_sb[:, t_i * 512:(t_i + 1) * 512],
                                sc3[:, :, 0:2 * F],
                                mybir.ActivationFunctionType.Exp,
                                scale=SCALE,
                            )

                        # ---- attn @ V' (+ denominator via ones col) ----
                        acol = b * 512 + VCOLS + D  # attn region in vra_ps
                        for h in range(H):
                            cbi = HORD.index(h)
                            nc.tensor.matmul(
                                vra_ps[:, acol + h * (DH + 1):
                                       acol + (h + 1) * (DH + 1)],
                                lhsT=et_sb[:, cbi * F:(cbi + 1) * F],
                                rhs=v_sb[:, (b * H + h) * (DH + 1):
                                         (b * H + h + 1) * (DH + 1)],
                                start=True, stop=True,
                            )

                        # per-batch reciprocal of the denominators (keeps it
                        # off the pair-end critical chain)
                        denoms = bass.AP(
                            tensor=vra_ps.tensor,
                            offset=vra_ps.offset + b * 512 + (VCOLS + D + DH),
                            ap=[vra_ps.ap[0], [DH + 1, H]],
                        )
                        nc.vector.reciprocal(rc_pair[:, b * H:(b + 1) * H], denoms)

                    # ---- pair-level tail: out = attn * recip_bcast + R ----
                    attn_v = bass.AP(
                        tensor=vra_ps.tensor,
                        offset=vra_ps.offset + (VCOLS + D),
                        ap=[vra_ps.ap[0], [512, PAIR], [DH + 1, H], [1, DH]],
                    )
                    rc_bc = bass.AP(
                        tensor=rc_pair.tensor, offset=rc_pair.offset,
                        ap=[rc_pair.ap[0], [1, PAIR * H], [0, DH]],
                    )
                    o1 = smalls.tile([F, PAIR * D], f32)
                    nc.vector.tensor_mul(o1, attn_v, rc_bc)
                    r_ap = bass.AP(
                        tensor=vra_ps.tensor,
                        offset=vra_ps.offset + VCOLS,
                        ap=[vra_ps.ap[0], [512, PAIR], [1, D]],
                    )
                    nc.vector.tensor_add(
                        ow[:, p * PAIR * D:(p + 1) * PAIR * D], o1, r_ap
                    )

                nc.sync.dma_start(out=out[:, w * GIO:(w + 1) * GIO, :], in_=ow)

    return nc


def _pad_qk(Wx: np.ndarray) -> np.ndarray:
    """[D, 128] -> [D, 256]: A/B groups of 4 heads at 32-aligned columns."""
    o = np.zeros((D, 2 * D), dtype=np.float32)
    for h in range(H):
        grp, s = divmod(h, 4)
        o[:, grp * D + s * 32:grp * D + s * 32 + DH] = Wx[:, h * DH:(h + 1) * DH]
    return o


def prep_in_maps(inputs_dict):
    inputs = np.asarray(inputs_dict["inputs"])
    W_query = np.asarray(inputs_dict["W_query"], dtype=np.float32)
    W_key = np.asarray(inputs_dict["W_key"], dtype=np.float32)
    W_value = np.asarray(inputs_dict["W_value"], dtype=np.float32)
    W_res = np.asarray(inputs_dict["W_res"], dtype=np.float32)

    xt_all = np.ascontiguousarray(inputs.transpose(2, 0, 1)).astype(BF16)
    wqk_np = np.concatenate([_pad_qk(W_query), _pad_qk(W_key)], axis=1).astype(BF16)
    wv_pad = np.zeros((D, VCOLS), dtype=np.float32)
    for h in range(H):
        wv_pad[:, h * (DH + 1):h * (DH + 1) + DH] = W_value[:, h * DH:(h + 1) * DH]
    wvr_np = np.concatenate([wv_pad, W_res], axis=1).astype(BF16)

    return [
        {
            "xt": np.ascontiguousarray(xt_all[:, c * BPC:(c + 1) * BPC, :]),
            "wqk": wqk_np,
            "wvr": wvr_np,
        }
        for c in range(N_CORES)
    ]


_COMPILED = {}


def _get_compiled():
    if "nc" not in _COMPILED:
        nc = bacc.Bacc(
            "TRN2", target_bir_lowering=False, debug=False, num_devices=N_CORES
        )
        build_kernel(nc)
        nc.compile()
        _COMPILED["nc"] = nc
    return _COMPILED["nc"]


def kernel(inputs, W_query, W_key, W_value, W_res, **kw):
    in_maps = prep_in_maps({
        "inputs": inputs, "W_query": W_query, "W_key": W_key,
        "W_value": W_value, "W_res": W_res,
    })
    nc = _get_compiled()
    res = run_bass_kernel_spmd(nc, in_maps, core_ids=list(range(N_CORES)))
    parts = [r["out"].transpose(1, 0, 2) for r in res.results]
    return np.concatenate(parts, axis=0)


if __name__ == "__main__":
    rng = np.random.default_rng(0)
    inp = {
        "inputs": rng.standard_normal((B, F, D)).astype(np.float32),
        "W_query": (rng.standard_normal((D, D)) * 0.05).astype(np.float32),
        "W_key": (rng.standard_normal((D, D)) * 0.05).astype(np.float32),
        "W_value": (rng.standard_normal((D, D)) * 0.05).astype(np.float32),
        "W_res": (rng.standard_normal((D, D)) * 0.05).astype(np.float32),
    }
    o = kernel(**inp)
    print("out shape", o.shape, o.dtype)


# revision 18
# speedup vs baseline: 51.2240x; 51.2240x over previous
"""Multi-head attention kernel for Trainium2 (Bass/Tile), 8-core data-parallel.

Problem: B=1024 batches of F=128 tokens, D=128 features, H=8 heads, dh=16.
  out = softmax(X Wq (X Wk)^T / sqrt(D)) (X Wv) + X Wr   (per head, concat)

Per-core structure (128 batches):
  - Host pre-transposes X to XT [D, B, F] bf16 (no on-device transpose,
    halved input DMA). All matmuls bf16 (fp32 matmul is 4x slower on PE).
  - Wq/Wk host-padded into A (heads 0-3) / B (heads 4-7) tiles with each
    head's 16 cols at a 32-aligned offset + 16 zero cols -> score matmuls
    are K=32 row-tiled matmuls at legal 32-aligned bases.
  - PSUM same-bank rule: matmuls with different row bases must not share a
    PSUM bank. Scores go to two 2-bank tiles X (bases 0/32 = head pairs
    {0,4},{1,5}) and Y (bases 64/96 = {2,6},{3,7}), one base per bank.
  - exp on ScalarE: two strided [128,512] instructions per batch (X then Y),
    scale=1/sqrt(D) fused, bf16 out feeds attn@V as stationary operand.
    Softmax max-subtraction skipped (|scores|/sqrt(D) < ~1).
  - attn@V: lhsT = expT_h [k,q], rhs = V'_h (16 value cols + 1 ones col) ->
    matmul emits unnormalized output AND the softmax denominator.
  - V'(+ones slot), R, and attn output share one PSUM bank per batch
    (all base-0 matmuls); pair-level tile for 2 batches (2 banks).
  - Tail on VectorE at pair granularity: recip(denoms), broadcast-expand
    (GpSimd), out = attn_unnorm * recip + R.
  - Output staged [F, B, E]; host transposes back.
"""

import numpy as np
import ml_dtypes

import concourse.bass as bass
import concourse.mybir as mybir
import concourse.tile as tile
from concourse import bacc
from concourse.bass_utils import run_bass_kernel_spmd

BF16 = ml_dtypes.bfloat16

N_CORES = 8
B, F, D = 1024, 128, 128
H, DH = 8, 16
BPC = B // N_CORES   # 128 batches per core
GIO = 8              # batches per IO wave (DMA granularity)
PAIR = 2             # batches per projection/tail pair
VCOLS = H * (DH + 1)  # 136
VRA = VCOLS + D + VCOLS  # 408: [V' 0:136 | R 136:264 | attn-out 264:400]
SCALE = 1.0 / float(D) ** 0.5
# et column-block order: X tile = heads 0,4,1,5; Y tile = heads 2,6,3,7
HORD = [0, 4, 1, 5, 2, 6, 3, 7]


def build_kernel(nc: bass.Bass):
    f32 = mybir.dt.float32
    bf16 = mybir.dt.bfloat16

    xt = nc.dram_tensor("xt", [D, BPC, F], bf16, kind="ExternalInput")
    # [WqA | WqB | WkA | WkB], each [D, 128], heads at 32-aligned cols
    wqk = nc.dram_tensor("wqk", [D, 4 * D], bf16, kind="ExternalInput")
    # [WvPad (136 cols, ones-slots zero) | Wr (128)]
    wvr = nc.dram_tensor("wvr", [D, VCOLS + D], bf16, kind="ExternalInput")
    out = nc.dram_tensor("out", [F, BPC, D], f32, kind="ExternalOutput")

    with tile.TileContext(nc) as tc:
        with (
            tc.tile_pool(name="singles", bufs=1) as singles,
            tc.tile_pool(name="xtp", bufs=2) as xtp,
            tc.tile_pool(name="qksb", bufs=2) as qksb,
            tc.tile_pool(name="etp", bufs=2) as etp,
            tc.tile_pool(name="vp", bufs=3) as vp,
            tc.tile_pool(name="smalls", bufs=3) as smalls,
            tc.tile_pool(name="outp", bufs=2) as outp,
            tc.tile_pool(name="qkps", bufs=2, space="PSUM") as qkps_pool,
            tc.tile_pool(name="scxp", bufs=1, space="PSUM") as scxp_pool,
            tc.tile_pool(name="scyp", bufs=1, space="PSUM") as scyp_pool,
            tc.tile_pool(name="vraps", bufs=1, space="PSUM") as vraps_pool,
        ):
            wqk_sb = singles.tile([D, 4 * D], bf16)
            wvr_sb = singles.tile([D, VCOLS + D], bf16)
            nc.sync.dma_start(out=wqk_sb, in_=wqk[:, :])
            nc.sync.dma_start(out=wvr_sb, in_=wvr[:, :])

            for w in range(BPC // GIO):  # 16 IO waves
                xtw = xtp.tile([D, GIO * F], bf16)
                nc.sync.dma_start(out=xtw, in_=xt[:, w * GIO:(w + 1) * GIO, :])
                ow = outp.tile([F, GIO * D], f32)

                def emit_qk(gbl):
                    # QT/KT projection for one batch: [QA|QB|KA|KB] x 128
                    xtb = xtw[:, gbl * F:(gbl + 1) * F]
                    qk_ps = qkps_pool.tile([D, 4 * F], f32)  # 1 bank
                    for i in range(4):
                        nc.tensor.matmul(
                            qk_ps[:, i * F:(i + 1) * F],
                            lhsT=wqk_sb[:, i * D:(i + 1) * D],
                            rhs=xtb,
                            start=True, stop=True,
                        )
                    qk_sb = qksb.tile([D, 4 * F], bf16)
                    nc.vector.tensor_copy(qk_sb, qk_ps)
                    return qk_sb

                # software-pipeline qk one pair ahead within the wave
                qk_pend = [emit_qk(0), emit_qk(1)]

                for p in range(GIO // PAIR):  # 4 pairs per wave
                    qk_sbs = qk_pend
                    qk_pend = []

                    # ---- V'/R projections for both batches of the pair ----
                    # [128, 1024] = 2 banks; batch b at cols 512*b + [0:400]
                    vra_ps = vraps_pool.tile([F, 2 * 512], f32)
                    for b in range(PAIR):
                        gb = p * PAIR + b
                        xtb = xtw[:, gb * F:(gb + 1) * F]
                        nc.tensor.matmul(
                            vra_ps[:, b * 512:b * 512 + VCOLS + D],
                            lhsT=xtb, rhs=wvr_sb, start=True, stop=True,
                        )
                    v_sb = vp.tile([F, PAIR * VCOLS], bf16)
                    v4 = v_sb.rearrange("p (b h c) -> p b h c", b=PAIR, c=DH + 1)
                    vrav = bass.AP(
                        tensor=vra_ps.tensor, offset=vra_ps.offset,
                        ap=[vra_ps.ap[0], [512, PAIR], [DH + 1, H], [1, DH]],
                    )
                    nc.gpsimd.memset(v4[:, :, :, DH:DH + 1], 1.0)
                    nc.vector.tensor_copy(v4[:, :, :, 0:DH], vrav)
                    rc_pair = smalls.tile([F, PAIR * H], f32, tag="rc")

                    for b in range(PAIR):
                        gb = p * PAIR + b
                        qk_sb = qk_sbs[b]
                        qtA = qk_sb[:, 0 * F:1 * F]
                        qtB = qk_sb[:, 1 * F:2 * F]
                        ktA = qk_sb[:, 2 * F:3 * F]
                        ktB = qk_sb[:, 3 * F:4 * F]

                        # ---- scores into X (bases 0/32), Y (bases 64/96);
                        # emit all X heads, then Y, so exp-X can start while
                        # PE fills Y ----
                        sc_x = scxp_pool.tile([F, 1024], f32)  # 2 banks
                        sc_y = scyp_pool.tile([F, 1024], f32)  # 2 banks
                        for h in HORD:
                            qt = qtA if h < 4 else qtB
                            kt = ktA if h < 4 else ktB
                            s = (h % 4) * 32
                            sc = sc_x if (h % 4) < 2 else sc_y
                            col = ((h % 4) % 2) * 512 + (h // 4) * F
                            nc.tensor.matmul(
                                sc[:, col:col + F],
                                lhsT=kt[s:s + 32, :],
                                rhs=qt[s:s + 32, :],
                                start=True, stop=True,
                                tile_position=(s, 0),
                            )

                        # ---- exp: one strided instr per sc tile ----
                        et_sb = etp.tile([F, H * F], bf16)
                        for t_i, sc in enumerate((sc_x, sc_y)):
                            sc3 = sc.rearrange("p (bk c) -> p bk c", bk=2)
                            nc.scalar.activation(
                                et_sb[:, t_i * 512:(t_i + 1) * 512],
                                sc3[:, :, 0:2 * F],
                                mybir.ActivationFunctionType.Exp,
                                scale=SCALE,
                            )

                        # emit next pair's qk AFTER this batch's scores/exp
                        # so it schedules into the exp window instead of
                        # delaying the next scores
                        if gb + PAIR < GIO:
                            qk_pend.append(emit_qk(gb + PAIR))

                        # ---- attn @ V' (+ denominator via ones col) ----
                        acol = b * 512 + VCOLS + D  # attn region in vra_ps
                        for h in range(H):
                            cbi = HORD.index(h)
                            nc.tensor.matmul(
                                vra_ps[:, acol + h * (DH + 1):
                                       acol + (h + 1) * (DH + 1)],
                                lhsT=et_sb[:, cbi * F:(cbi + 1) * F],
                                rhs=v_sb[:, (b * H + h) * (DH + 1):
                                         (b * H + h + 1) * (DH + 1)],
                                start=True, stop=True,
                            )

                        # per-batch reciprocal of the denominators (keeps it
                        # off the pair-end critical chain)
                        denoms = bass.AP(
                            tensor=vra_ps.tensor,
                            offset=vra_ps.offset + b * 512 + (VCOLS + D + DH),
                            ap=[vra_ps.ap[0], [DH + 1, H]],
                        )
                        nc.vector.reciprocal(rc_pair[:, b * H:(b + 1) * H], denoms)

                    # ---- pair-level tail: out = attn * recip_bcast + R ----
                    attn_v = bass.AP(
                        tensor=vra_ps.tensor,
                        offset=vra_ps.offset + (VCOLS + D),
                        ap=[vra_ps.ap[0], [512, PAIR], [DH + 1, H], [1, DH]],
                    )
                    rc_bc = bass.AP(
                        tensor=rc_pair.tensor, offset=rc_pair.offset,
                        ap=[rc_pair.ap[0], [1, PAIR * H], [0, DH]],
                    )
                    o1 = smalls.tile([F, PAIR * D], f32)
                    nc.vector.tensor_mul(o1, attn_v, rc_bc)
                    r_ap = bass.AP(
                        tensor=vra_ps.tensor,
                        offset=vra_ps.offset + VCOLS,
                        ap=[vra_ps.ap[0], [512, PAIR], [1, D]],
                    )
                    nc.vector.tensor_add(
                        ow[:, p * PAIR * D:(p + 1) * PAIR * D], o1, r_ap
                    )

                nc.sync.dma_start(out=out[:, w * GIO:(w + 1) * GIO, :], in_=ow)

    return nc


def _pad_qk(Wx: np.ndarray) -> np.ndarray:
    """[D, 128] -> [D, 256]: A/B groups of 4 heads at 32-aligned columns."""
    o = np.zeros((D, 2 * D), dtype=np.float32)
    for h in range(H):
        grp, s = divmod(h, 4)
        o[:, grp * D + s * 32:grp * D + s * 32 + DH] = Wx[:, h * DH:(h + 1) * DH]
    return o


def prep_in_maps(inputs_dict):
    inputs = np.asarray(inputs_dict["inputs"])
    W_query = np.asarray(inputs_dict["W_query"], dtype=np.float32)
    W_key = np.asarray(inputs_dict["W_key"], dtype=np.float32)
    W_value = np.asarray(inputs_dict["W_value"], dtype=np.float32)
    W_res = np.asarray(inputs_dict["W_res"], dtype=np.float32)

    xt_all = np.ascontiguousarray(inputs.transpose(2, 0, 1)).astype(BF16)
    wqk_np = np.concatenate([_pad_qk(W_query), _pad_qk(W_key)], axis=1).astype(BF16)
    wv_pad = np.zeros((D, VCOLS), dtype=np.float32)
    for h in range(H):
        wv_pad[:, h * (DH + 1):h * (DH + 1) + DH] = W_value[:, h * DH:(h + 1) * DH]
    wvr_np = np.concatenate([wv_pad, W_res], axis=1).astype(BF16)

    return [
        {
            "xt": np.ascontiguousarray(xt_all[:, c * BPC:(c + 1) * BPC, :]),
            "wqk": wqk_np,
            "wvr": wvr_np,
        }
        for c in range(N_CORES)
    ]


_COMPILED = {}


def _get_compiled():
    if "nc" not in _COMPILED:
        nc = bacc.Bacc(
            "TRN2", target_bir_lowering=False, debug=False, num_devices=N_CORES
        )
        build_kernel(nc)
        nc.compile()
        _COMPILED["nc"] = nc
    return _COMPILED["nc"]


def kernel(inputs, W_query, W_key, W_value, W_res, **kw):
    in_maps = prep_in_maps({
        "inputs": inputs, "W_query": W_query, "W_key": W_key,
        "W_value": W_value, "W_res": W_res,
    })
    nc = _get_compiled()
    res = run_bass_kernel_spmd(nc, in_maps, core_ids=list(range(N_CORES)))
    parts = [r["out"].transpose(1, 0, 2) for r in res.results]
    return np.concatenate(parts, axis=0)


if __name__ == "__main__":
    rng = np.random.default_rng(0)
    inp = {
        "inputs": rng.standard_normal((B, F, D)).astype(np.float32),
        "W_query": (rng.standard_normal((D, D)) * 0.05).astype(np.float32),
        "W_key": (rng.standard_normal((D, D)) * 0.05).astype(np.float32),
        "W_value": (rng.standard_normal((D, D)) * 0.05).astype(np.float32),
        "W_res": (rng.standard_normal((D, D)) * 0.05).astype(np.float32),
    }
    o = kernel(**inp)
    print("out shape", o.shape, o.dtype)
